# revision 20
# baseline (speedup 1.0000x reference)
"""Multi-head attention (b=2, t=2048, d=1024, h=16, hd=64) on 8 trn2 NeuronCores.

Sharding: core c = 4*b + g handles batch b and head-group g (4 heads,
feature columns [g*256, (g+1)*256)). QKV weights column-sharded, Wo
row-sharded (Megatron); each core returns two partial [2048, 1024]
outputs (head-pair 0 / head-pair 1 of its group) that the host sums,
plus bo.

Datapath: fp16 operands (x, Wq/Wk/Wv, Q^T, K^T, V, probs) with fp32
PSUM accumulation everywhere; the context normalize and output
projection run in f32r (TF32-class). Softmax skips max-subtraction:
scores are q.k/8 with q,k ~ N(0,1), far inside exp's range.

Attention is ACT(exp)-bound, so PE work from other phases (Q/K fb1
projections, V transposes via DMA-xbar, pair-0 output projection) is
interleaved into the attention loops to keep the PE HAM clock-gate
warm (cold K=4/8 halves the PE clock).
"""

import numpy as np

import concourse.bass as bass
import concourse.mybir as mybir
import concourse.tile as tile
from concourse.bass_utils import run_bass_kernel_spmd
from concourse.masks import make_identity

F32 = mybir.dt.float32
F32R = mybir.dt.float32r
F16 = mybir.dt.float16
EXP = mybir.ActivationFunctionType.Exp

T = 2048          # tokens per batch
D = 1024          # model dim
HG = 4            # heads per core
HD = 64           # head dim
GF = HG * HD      # 256 features per head-group
VW = HG * (HD + 1)  # 260: V columns + a ones column per head
NT = T // 128     # 16 token blocks

MAX_WAITS = 1


def _split_waits(nc):
    """walrus in this container allows only one sync-wait per instruction;
    hoist extras onto same-engine NoOps immediately before the offender."""
    for f in nc.m.functions:
        for blk in f.blocks:
            insts = list(blk.instructions)
            new, changed = [], False
            for ins in insts:
                si = ins.sync_info
                waits = list(si.on_wait) if si and si.on_wait else []
                if len(waits) > MAX_WAITS:
                    changed = True
                    extra, keep = waits[:-MAX_WAITS], waits[-MAX_WAITS:]
                    for i in range(0, len(extra), MAX_WAITS):
                        new.append(mybir.InstNoOp(
                            name=f"{ins.name}-wsplit{i}",
                            engine=ins.engine,
                            sync_info=mybir.SyncInfo(
                                on_wait=extra[i:i + MAX_WAITS], on_update=[]),
                        ))
                    ins.sync_info = mybir.SyncInfo(
                        on_wait=keep,
                        on_update=list(si.on_update) if si.on_update else [])
                new.append(ins)
            if changed:
                blk.instructions = new


def _build_program():
    nc = bass.Bass("TRN2", target_bir_lowering=False, debug=False, num_devices=8)

    xT = nc.dram_tensor("xT", [D, T], F16, kind="ExternalInput")
    Wq = nc.dram_tensor("Wq", [D, GF], F16, kind="ExternalInput")
    Wk = nc.dram_tensor("Wk", [D, GF], F16, kind="ExternalInput")
    Wv = nc.dram_tensor("Wv", [D, GF], F16, kind="ExternalInput")
    Wo = nc.dram_tensor("Wo", [GF, D], F32R, kind="ExternalInput")
    bq = nc.dram_tensor("bq", [GF, 1], F32, kind="ExternalInput")
    bk = nc.dram_tensor("bk", [GF, 1], F32, kind="ExternalInput")
    bv = nc.dram_tensor("bv", [GF, 1], F32, kind="ExternalInput")
    # single output holding both head-pair partials: [pair*T + t, D]
    out = nc.dram_tensor("out", [2 * T, D], F32, kind="ExternalOutput")

    with tile.TileContext(nc) as tc:
        with (
            nc.allow_low_precision(reason="fp16/f32r rounding is intentional"),
            tc.tile_pool(name="w", bufs=1) as wp,       # persistent tiles
            tc.tile_pool(name="xt", bufs=8) as xp,      # xT tiles
            tc.tile_pool(name="pt", bufs=4) as ptp,     # probs tiles
            tc.tile_pool(name="ob", bufs=2) as obp,     # out staging
            tc.tile_pool(name="ps", bufs=2, space="PSUM") as ps,    # "sp" slots
            tc.tile_pool(name="pst", bufs=2, space="PSUM") as pst,  # S tiles
            tc.tile_pool(name="psc", bufs=1, space="PSUM") as psc,  # C accum
        ):
            # ---- input DMAs ------------------------------------------------
            xT_t, Wq_t, Wk_t, Wv_t = [], [], [], []
            for dc in range(8):
                xt = xp.tile([128, T], F16, tag="xt")
                nc.sync.dma_start(xt[:], xT[dc * 128:(dc + 1) * 128, :])
                xT_t.append(xt)
                for (lst, src, nm) in ((Wq_t, Wq, "wq"), (Wk_t, Wk, "wk"),
                                       (Wv_t, Wv, "wv")):
                    w = wp.tile([128, GF], F16, tag=f"{nm}{dc}", name=f"{nm}{dc}")
                    nc.sync.dma_start(w[:], src[dc * 128:(dc + 1) * 128, :])
                    lst.append(w)
            Wo_t = []
            for pair in range(2):
                wo = wp.tile([128, D], F32R, tag=f"wo{pair}", name=f"wo{pair}")
                nc.sync.dma_start(wo[:], Wo[pair * 128:(pair + 1) * 128, :])
                Wo_t.append(wo)
            bq_t, bk_t, bv_t = [], [], []
            for fb in range(2):
                for (lst, src, nm) in ((bq_t, bq, "bq"), (bk_t, bk, "bk"),
                                       (bv_t, bv, "bv")):
                    b = wp.tile([128, 1], F32, tag=f"{nm}{fb}", name=f"{nm}{fb}")
                    nc.sync.dma_start(b[:], src[fb * 128:(fb + 1) * 128, :])
                    lst.append(b)

            # ones row living at base partition 64, to pair with the
            # denominator row (psum row 64) in the replicate matmul
            ones_f = wp.tile([65, 128], F32, tag="ones_f")
            nc.gpsimd.memset(ones_f[:], 1.0)
            onesr = wp.tile([65, 128], F32R, tag="onesr")
            nc.vector.tensor_copy(onesr[:], ones_f[:])

            # ---- projection helpers (feature-major: [feat, tokens]) --------
            QT = [wp.tile([128, T], F16, tag=f"qt{fb}", name=f"qt{fb}")
                  for fb in range(2)]
            KT = [wp.tile([128, T], F16, tag=f"kt{fb}", name=f"kt{fb}")
                  for fb in range(2)]
            VT = [wp.tile([128, T], F16, tag=f"vt{fb}", name=f"vt{fb}")
                  for fb in range(2)]

            def proj_group(w_t, b_t, dst, fb, tck):
                p = ps.tile([128, 512], F32, tag="sp", name="sp")
                for dc in range(8):
                    nc.tensor.matmul(
                        p[:],
                        w_t[dc][:, fb * 128:(fb + 1) * 128],
                        xT_t[dc][:, tck * 512:(tck + 1) * 512],
                        start=(dc == 0), stop=(dc == 7))
                nc.vector.tensor_scalar_add(
                    dst[fb][:, tck * 512:(tck + 1) * 512], p[:], b_t[fb])

            # V_t[sb]: token-major [128 tokens, 4*(64+1)] with ones columns
            V_t = [wp.tile([128, VW], F16, tag=f"v{tb}", name=f"v{tb}")
                   for tb in range(NT)]

            ident = wp.tile([128, 128], F16, tag="ident")
            make_identity(nc, ident[:])

            def vt_build(sb):
                # cols h*65..h*65+63 hold V; col h*65+64 is a ones column
                # so the C psum row 64 is the softmax denominator.
                for h in range(HG):
                    nc.gpsimd.memset(V_t[sb][:, h * 65 + 64:h * 65 + 65], 1.0)
                for fb in range(2):
                    tp = ps.tile([128, 128], F16, tag="sp", name="sp")
                    nc.tensor.transpose(
                        tp[:],
                        VT[fb][:, sb * 128:(sb + 1) * 128], ident[:])
                    for hh in range(2):
                        h = fb * 2 + hh
                        nc.vector.tensor_copy(
                            V_t[sb][:, h * 65:h * 65 + 64],
                            tp[:, hh * 64:hh * 64 + 64])

            CTn = [wp.tile([128, T], F32R, tag=f"ctn{p}", name=f"ctn{p}")
                   for p in range(2)]

            # pair-`pair` partial output projection for token block tb
            def out_unit(pair, tb):
                o = obp.tile([128, D], F32, tag="o", name="o")
                for nck in range(2):
                    p = ps.tile([128, 512], F32, tag="sp", name="sp")
                    nc.tensor.matmul(
                        p[:],
                        CTn[pair][:, tb * 128:(tb + 1) * 128],
                        Wo_t[pair][:, nck * 512:(nck + 1) * 512],
                        start=True, stop=True)
                    nc.vector.tensor_copy(o[:, nck * 512:(nck + 1) * 512], p[:])
                nc.sync.dma_start(
                    out[pair * T + tb * 128:pair * T + (tb + 1) * 128, :], o[:])

            # ---- pre-phase: Q/K fb0 and all of V^T -------------------------
            for tck in range(4):
                proj_group(Wq_t, bq_t, QT, 0, tck)
            for tck in range(4):
                proj_group(Wk_t, bk_t, KT, 0, tck)
            for fb in range(2):
                for tck in range(4):
                    proj_group(Wv_t, bv_t, VT, fb, tck)

            # ---- attention: 4 heads x 2 token-halves -----------------------
            # filler schedule per (h, half) pass: PE/DMA work from other
            # phases, interleaved to keep the PE busy while ACT runs exp.
            qk1 = ([lambda t=t: proj_group(Wq_t, bq_t, QT, 1, t)
                    for t in range(4)] +
                   [lambda t=t: proj_group(Wk_t, bk_t, KT, 1, t)
                    for t in range(4)])
            def warm_unit():
                # tiny matmul with no upstream deps: keeps the PE HAM
                # activity monitor from re-throttling during ACT-bound
                # stretches that have no real filler work.
                p = ps.tile([128, 512], F32, tag="sp", name="sp")
                nc.tensor.matmul(p[0:1, :], KT[0][0:1, 0:1], KT[0][0:1, 0:512],
                                 start=True, stop=True)

            fillers = {
                (0, 0): [lambda j=j: vt_build(j) for j in range(NT)],
                (0, 1): [qk1[j // 4] if j % 4 == 0 else None for j in range(16)],
                (1, 0): [qk1[4 + j // 4] if j % 4 == 0 else None
                         for j in range(16)],
                (1, 1): [(lambda t=(j // 2): out_unit(0, t)) if j % 2 == 0
                         else None for j in range(16)],
                (2, 0): [(lambda t=(8 + j // 2): out_unit(0, t)) if j % 2 == 0
                         else None for j in range(16)],
                (2, 1): [warm_unit for j in range(16)],
                (3, 0): [warm_unit for j in range(16)],
                (3, 1): [(lambda t=(j // 2): out_unit(1, t)) if j % 2 == 0
                         else None for j in range(16)],
            }

            for h in range(HG):
                fb, ro = h // 2, (h % 2) * 64
                for half in range(2):
                    hc = half * 1024
                    ct = psc.tile([65, 1024], F32, tag="ct", name="ct")
                    pts = {}
                    fl = fillers.get((h, half), [])

                    def c_mms(j, ct=ct, h=h, pts=pts):
                        for q in range(2):
                            nc.tensor.matmul(
                                ct[:, q * 512:(q + 1) * 512],
                                V_t[j][:, h * 65:(h + 1) * 65],
                                pts[j][:, q * 512:(q + 1) * 512],
                                start=(j == 0), stop=(j == NT - 1))

                    for sb in range(NT):
                        pt = ptp.tile([128, 1024], F16, tag="pt", name="pt")
                        pts[sb] = pt
                        st = pst.tile([128, 1024], F32, tag="st", name="st")
                        for q in range(2):
                            nc.tensor.matmul(
                                st[:, q * 512:(q + 1) * 512],
                                KT[fb][ro:ro + 64, sb * 128:(sb + 1) * 128],
                                QT[fb][ro:ro + 64,
                                       hc + q * 512:hc + (q + 1) * 512],
                                start=True, stop=True)
                        nc.scalar.activation(pt[:], st[:], EXP, scale=0.125)
                        if sb < len(fl) and fl[sb] is not None:
                            fl[sb]()
                        if sb > 0:
                            c_mms(sb - 1)
                    c_mms(NT - 1)

                    # free ct fast: stage raw C + denominator to SBUF,
                    # then normalize off the critical path.
                    stg = wp.tile([65, 1024], F32R, tag=f"stg{half}",
                                  name=f"stg{half}")
                    nc.vector.tensor_copy(stg[:], ct[:])
                    for q in range(2):
                        rp = ps.tile([128, 512], F32, tag="sp", name="sp")
                        nc.tensor.matmul(
                            rp[:], onesr[64:65, :],
                            stg[64:65, q * 512:(q + 1) * 512],
                            start=True, stop=True)
                        rb = wp.tile([64, 512], F32, tag=f"rb{q}", name=f"rb{q}")
                        nc.vector.reciprocal(rb[:], rp[0:64, :])
                        nc.vector.tensor_mul(
                            CTn[fb][ro:ro + 64,
                                    hc + q * 512:hc + (q + 1) * 512],
                            stg[0:64, q * 512:(q + 1) * 512],
                            rb[:])

            # ---- remaining pair-1 output projection ------------------------
            for tb in range(8, NT):
                out_unit(1, tb)

    _split_waits(nc)
    return nc


_NC = None


def _get_nc():
    global _NC
    if _NC is None:
        _NC = _build_program()
    return _NC


def _shard_inputs(x, Wq, bq, Wk, bk, Wv, bv, Wo):
    xTs = [np.ascontiguousarray(x[b].T).astype(np.float16) for b in range(2)]
    in_maps = []
    for core in range(8):
        b, g = divmod(core, 4)
        lo = g * GF
        in_maps.append({
            "xT": xTs[b],
            "Wq": np.ascontiguousarray(Wq[:, lo:lo + GF]).astype(np.float16),
            "Wk": np.ascontiguousarray(Wk[:, lo:lo + GF]).astype(np.float16),
            "Wv": np.ascontiguousarray(Wv[:, lo:lo + GF]).astype(np.float16),
            "Wo": np.ascontiguousarray(Wo[lo:lo + GF, :]),
            "bq": np.ascontiguousarray(bq[lo:lo + GF].reshape(GF, 1)),
            "bk": np.ascontiguousarray(bk[lo:lo + GF].reshape(GF, 1)),
            "bv": np.ascontiguousarray(bv[lo:lo + GF].reshape(GF, 1)),
        })
    return in_maps


def run(inputs, trace=False, trace_kwargs=None):
    """Run the kernel; returns (output [2,2048,1024] f32, BassKernelResults)."""
    inputs = {k: np.asarray(v, dtype=np.float32) for k, v in inputs.items()}
    in_maps = _shard_inputs(
        inputs["x"], inputs["Wq"], inputs["bq"], inputs["Wk"], inputs["bk"],
        inputs["Wv"], inputs["bv"], inputs["Wo"])
    nc = _get_nc()
    res = run_bass_kernel_spmd(
        nc, in_maps, list(range(8)), trace=trace, **(trace_kwargs or {}))
    bo = inputs["bo"]
    out = np.empty((2, T, D), dtype=np.float32)
    for b in range(2):
        acc = None
        for g in range(4):
            part = res.results[4 * b + g]["out"]
            for pair in range(2):
                piece = part[pair * T:(pair + 1) * T]
                acc = piece.astype(np.float32).copy() if acc is None else acc + piece
        out[b] = acc + bo[None, :]
    return out, res


def kernel(**inputs):
    out, _ = run(inputs, trace=False)
    return out


# revision 21
# speedup vs baseline: 1.1273x; 1.1273x over previous
"""Multi-head attention (b=2, t=2048, d=1024, h=16, hd=64) on 8 trn2 NeuronCores.

Sharding: core c = 4*b + g handles batch b and head-group g (4 heads,
feature columns [g*256, (g+1)*256)). QKV weights column-sharded, Wo
row-sharded (Megatron); each core returns two partial [2048, 1024]
outputs (head-pair 0 / head-pair 1 of its group) that the host sums,
plus bo.

Datapath: fp16 operands (x, Wq/Wk/Wv, Q^T, K^T, V, probs) with fp32
PSUM accumulation everywhere; the context normalize and output
projection run in f32r (TF32-class). Softmax skips max-subtraction:
scores are q.k/8 with q,k ~ N(0,1), far inside exp's range.

Attention is ACT(exp)-bound, so PE work from other phases (Q/K fb1
projections, V transposes via DMA-xbar, pair-0 output projection) is
interleaved into the attention loops to keep the PE HAM clock-gate
warm (cold K=4/8 halves the PE clock).
"""

import numpy as np

import concourse.bass as bass
import concourse.mybir as mybir
import concourse.tile as tile
from concourse.bass_utils import run_bass_kernel_spmd
from concourse.masks import make_identity

F32 = mybir.dt.float32
F32R = mybir.dt.float32r
F16 = mybir.dt.float16
EXP = mybir.ActivationFunctionType.Exp

T = 2048          # tokens per batch
D = 1024          # model dim
HG = 4            # heads per core
HD = 64           # head dim
GF = HG * HD      # 256 features per head-group
VW = HG * (HD + 1)  # 260: V columns + a ones column per head
NT = T // 128     # 16 token blocks

MAX_WAITS = 1


def _split_waits(nc):
    """walrus in this container allows only one sync-wait per instruction;
    hoist extras onto same-engine NoOps immediately before the offender."""
    for f in nc.m.functions:
        for blk in f.blocks:
            insts = list(blk.instructions)
            new, changed = [], False
            for ins in insts:
                si = ins.sync_info
                waits = list(si.on_wait) if si and si.on_wait else []
                if len(waits) > MAX_WAITS:
                    changed = True
                    extra, keep = waits[:-MAX_WAITS], waits[-MAX_WAITS:]
                    for i in range(0, len(extra), MAX_WAITS):
                        new.append(mybir.InstNoOp(
                            name=f"{ins.name}-wsplit{i}",
                            engine=ins.engine,
                            sync_info=mybir.SyncInfo(
                                on_wait=extra[i:i + MAX_WAITS], on_update=[]),
                        ))
                    ins.sync_info = mybir.SyncInfo(
                        on_wait=keep,
                        on_update=list(si.on_update) if si.on_update else [])
                new.append(ins)
            if changed:
                blk.instructions = new


def _build_program():
    nc = bass.Bass("TRN2", target_bir_lowering=False, debug=False, num_devices=8)

    xT = nc.dram_tensor("xT", [D, T], F16, kind="ExternalInput")
    Wq = nc.dram_tensor("Wq", [D, GF], F16, kind="ExternalInput")
    Wk = nc.dram_tensor("Wk", [D, GF], F16, kind="ExternalInput")
    Wv = nc.dram_tensor("Wv", [D, GF], F16, kind="ExternalInput")
    Wo = nc.dram_tensor("Wo", [GF, D], F32R, kind="ExternalInput")
    bq = nc.dram_tensor("bq", [GF, 1], F32, kind="ExternalInput")
    bk = nc.dram_tensor("bk", [GF, 1], F32, kind="ExternalInput")
    bv = nc.dram_tensor("bv", [GF, 1], F32, kind="ExternalInput")
    # single output holding both head-pair partials: [pair*T + t, D]
    out = nc.dram_tensor("out", [2 * T, D], F32, kind="ExternalOutput")

    with tile.TileContext(nc) as tc:
        with (
            nc.allow_low_precision(reason="fp16/f32r rounding is intentional"),
            tc.tile_pool(name="w", bufs=1) as wp,       # persistent tiles
            tc.tile_pool(name="xt", bufs=8) as xp,      # xT tiles
            tc.tile_pool(name="pt", bufs=4) as ptp,     # probs tiles
            tc.tile_pool(name="ob", bufs=2) as obp,     # out staging
            tc.tile_pool(name="ps", bufs=2, space="PSUM") as ps,    # "sp" slots
            tc.tile_pool(name="pst", bufs=2, space="PSUM") as pst,  # S tiles
            tc.tile_pool(name="psc", bufs=1, space="PSUM") as psc,  # C accum
        ):
            # ---- input DMAs ------------------------------------------------
            xT_t, Wq_t, Wk_t, Wv_t = [], [], [], []
            for dc in range(8):
                xt = xp.tile([128, T], F16, tag="xt")
                nc.sync.dma_start(xt[:], xT[dc * 128:(dc + 1) * 128, :])
                xT_t.append(xt)
                for (lst, src, nm) in ((Wq_t, Wq, "wq"), (Wk_t, Wk, "wk"),
                                       (Wv_t, Wv, "wv")):
                    w = wp.tile([128, GF], F16, tag=f"{nm}{dc}", name=f"{nm}{dc}")
                    nc.sync.dma_start(w[:], src[dc * 128:(dc + 1) * 128, :])
                    lst.append(w)
            Wo_t = []
            for pair in range(2):
                wo = wp.tile([128, D], F32R, tag=f"wo{pair}", name=f"wo{pair}")
                nc.sync.dma_start(wo[:], Wo[pair * 128:(pair + 1) * 128, :])
                Wo_t.append(wo)
            bq_t, bk_t, bv_t = [], [], []
            for fb in range(2):
                for (lst, src, nm) in ((bq_t, bq, "bq"), (bk_t, bk, "bk"),
                                       (bv_t, bv, "bv")):
                    b = wp.tile([128, 1], F32, tag=f"{nm}{fb}", name=f"{nm}{fb}")
                    nc.sync.dma_start(b[:], src[fb * 128:(fb + 1) * 128, :])
                    lst.append(b)

            # ones row living at base partition 64, to pair with the
            # denominator row (psum row 64) in the replicate matmul
            ones_f = wp.tile([65, 128], F32, tag="ones_f")
            nc.gpsimd.memset(ones_f[:], 1.0)
            onesr = wp.tile([65, 128], F32R, tag="onesr")
            nc.vector.tensor_copy(onesr[:], ones_f[:])

            # ---- projection helpers (feature-major: [feat, tokens]) --------
            QT = [wp.tile([128, T], F16, tag=f"qt{fb}", name=f"qt{fb}")
                  for fb in range(2)]
            KT = [wp.tile([128, T], F16, tag=f"kt{fb}", name=f"kt{fb}")
                  for fb in range(2)]
            VT = [wp.tile([128, T], F16, tag=f"vt{fb}", name=f"vt{fb}")
                  for fb in range(2)]

            def proj_group(w_t, b_t, dst, fb, tck):
                p = ps.tile([128, 512], F32, tag="sp", name="sp")
                for dc in range(8):
                    nc.tensor.matmul(
                        p[:],
                        w_t[dc][:, fb * 128:(fb + 1) * 128],
                        xT_t[dc][:, tck * 512:(tck + 1) * 512],
                        start=(dc == 0), stop=(dc == 7))
                nc.vector.tensor_scalar_add(
                    dst[fb][:, tck * 512:(tck + 1) * 512], p[:], b_t[fb])

            # V_t[sb]: token-major [128 tokens, 4*(64+1)] with ones columns
            V_t = [wp.tile([128, VW], F16, tag=f"v{tb}", name=f"v{tb}")
                   for tb in range(NT)]

            ident = wp.tile([128, 128], F16, tag="ident")
            make_identity(nc, ident[:])

            def vt_build(sb):
                # cols h*65..h*65+63 hold V; col h*65+64 is a ones column
                # so the C psum row 64 is the softmax denominator.
                for h in range(HG):
                    nc.gpsimd.memset(V_t[sb][:, h * 65 + 64:h * 65 + 65], 1.0)
                for fb in range(2):
                    tp = ps.tile([128, 128], F16, tag="sp", name="sp")
                    nc.tensor.transpose(
                        tp[:],
                        VT[fb][:, sb * 128:(sb + 1) * 128], ident[:])
                    for hh in range(2):
                        h = fb * 2 + hh
                        nc.vector.tensor_copy(
                            V_t[sb][:, h * 65:h * 65 + 64],
                            tp[:, hh * 64:hh * 64 + 64])

            CTn = [wp.tile([128, T], F32R, tag=f"ctn{p}", name=f"ctn{p}")
                   for p in range(2)]

            # pair-`pair` partial output projection for token block tb
            def out_unit(pair, tb):
                o = obp.tile([128, D], F32, tag="o", name="o")
                for nck in range(2):
                    p = ps.tile([128, 512], F32, tag="sp", name="sp")
                    nc.tensor.matmul(
                        p[:],
                        CTn[pair][:, tb * 128:(tb + 1) * 128],
                        Wo_t[pair][:, nck * 512:(nck + 1) * 512],
                        start=True, stop=True)
                    nc.vector.tensor_copy(o[:, nck * 512:(nck + 1) * 512], p[:])
                nc.sync.dma_start(
                    out[pair * T + tb * 128:pair * T + (tb + 1) * 128, :], o[:])

            # ---- pre-phase: Q/K fb0 and all of V^T -------------------------
            for tck in range(4):
                proj_group(Wq_t, bq_t, QT, 0, tck)
            for tck in range(4):
                proj_group(Wk_t, bk_t, KT, 0, tck)
            for fb in range(2):
                for tck in range(4):
                    proj_group(Wv_t, bv_t, VT, fb, tck)

            # ---- attention: 4 heads x 2 token-halves -----------------------
            # filler schedule per (h, half) pass: PE/DMA work from other
            # phases, interleaved to keep the PE busy while ACT runs exp.
            qk1 = ([lambda t=t: proj_group(Wq_t, bq_t, QT, 1, t)
                    for t in range(4)] +
                   [lambda t=t: proj_group(Wk_t, bk_t, KT, 1, t)
                    for t in range(4)])
            fillers = {
                (0, 0): [lambda j=j: vt_build(j) for j in range(NT)],
                (0, 1): [qk1[j // 4] if j % 4 == 0 else None for j in range(16)],
                (1, 0): [qk1[4 + j // 4] if j % 4 == 0 else None
                         for j in range(16)],
                (2, 0): [(lambda t=(j // 2): out_unit(0, t)) if j % 2 == 0
                         else None for j in range(16)],
                (2, 1): [(lambda t=(8 + j // 2): out_unit(0, t)) if j % 2 == 0
                         else None for j in range(16)],
                (3, 1): [(lambda t=(j // 2): out_unit(1, t)) if j % 2 == 0
                         else None for j in range(16)],
            }

            for h in range(HG):
                fb, ro = h // 2, (h % 2) * 64
                for half in range(2):
                    hc = half * 1024
                    ct = psc.tile([65, 1024], F32, tag="ct", name="ct")
                    pts = {}
                    fl = fillers.get((h, half), [])

                    def c_mms(j, ct=ct, h=h, pts=pts):
                        for q in range(2):
                            nc.tensor.matmul(
                                ct[:, q * 512:(q + 1) * 512],
                                V_t[j][:, h * 65:(h + 1) * 65],
                                pts[j][:, q * 512:(q + 1) * 512],
                                start=(j == 0), stop=(j == NT - 1))

                    for sb in range(NT):
                        pt = ptp.tile([128, 1024], F16, tag="pt", name="pt")
                        pts[sb] = pt
                        st = pst.tile([128, 1024], F32, tag="st", name="st")
                        for q in range(2):
                            nc.tensor.matmul(
                                st[:, q * 512:(q + 1) * 512],
                                KT[fb][ro:ro + 64, sb * 128:(sb + 1) * 128],
                                QT[fb][ro:ro + 64,
                                       hc + q * 512:hc + (q + 1) * 512],
                                start=True, stop=True)
                        nc.scalar.activation(pt[:], st[:], EXP, scale=0.125)
                        if sb < len(fl) and fl[sb] is not None:
                            fl[sb]()
                        if sb > 0:
                            c_mms(sb - 1)
                    c_mms(NT - 1)

                    # free ct fast: stage raw C + denominator to SBUF,
                    # then normalize off the critical path.
                    stg = wp.tile([65, 1024], F32R, tag=f"stg{half}",
                                  name=f"stg{half}")
                    nc.vector.tensor_copy(stg[:], ct[:])
                    for q in range(2):
                        rp = ps.tile([128, 512], F32, tag="sp", name="sp")
                        nc.tensor.matmul(
                            rp[:], onesr[64:65, :],
                            stg[64:65, q * 512:(q + 1) * 512],
                            start=True, stop=True)
                        rb = wp.tile([64, 512], F32, tag=f"rb{q}", name=f"rb{q}")
                        nc.vector.reciprocal(rb[:], rp[0:64, :])
                        nc.vector.tensor_mul(
                            CTn[fb][ro:ro + 64,
                                    hc + q * 512:hc + (q + 1) * 512],
                            stg[0:64, q * 512:(q + 1) * 512],
                            rb[:])

            # ---- remaining pair-1 output projection ------------------------
            for tb in range(8, NT):
                out_unit(1, tb)

    _split_waits(nc)
    return nc


_NC = None


def _get_nc():
    global _NC
    if _NC is None:
        _NC = _build_program()
    return _NC


def _shard_inputs(x, Wq, bq, Wk, bk, Wv, bv, Wo):
    xTs = [np.ascontiguousarray(x[b].T).astype(np.float16) for b in range(2)]
    in_maps = []
    for core in range(8):
        b, g = divmod(core, 4)
        lo = g * GF
        in_maps.append({
            "xT": xTs[b],
            "Wq": np.ascontiguousarray(Wq[:, lo:lo + GF]).astype(np.float16),
            "Wk": np.ascontiguousarray(Wk[:, lo:lo + GF]).astype(np.float16),
            "Wv": np.ascontiguousarray(Wv[:, lo:lo + GF]).astype(np.float16),
            "Wo": np.ascontiguousarray(Wo[lo:lo + GF, :]),
            "bq": np.ascontiguousarray(bq[lo:lo + GF].reshape(GF, 1)),
            "bk": np.ascontiguousarray(bk[lo:lo + GF].reshape(GF, 1)),
            "bv": np.ascontiguousarray(bv[lo:lo + GF].reshape(GF, 1)),
        })
    return in_maps


def run(inputs, trace=False, trace_kwargs=None):
    """Run the kernel; returns (output [2,2048,1024] f32, BassKernelResults)."""
    inputs = {k: np.asarray(v, dtype=np.float32) for k, v in inputs.items()}
    in_maps = _shard_inputs(
        inputs["x"], inputs["Wq"], inputs["bq"], inputs["Wk"], inputs["bk"],
        inputs["Wv"], inputs["bv"], inputs["Wo"])
    nc = _get_nc()
    res = run_bass_kernel_spmd(
        nc, in_maps, list(range(8)), trace=trace, **(trace_kwargs or {}))
    bo = inputs["bo"]
    out = np.empty((2, T, D), dtype=np.float32)
    for b in range(2):
        acc = None
        for g in range(4):
            part = res.results[4 * b + g]["out"]
            for pair in range(2):
                piece = part[pair * T:(pair + 1) * T]
                acc = piece.astype(np.float32).copy() if acc is None else acc + piece
        out[b] = acc + bo[None, :]
    return out, res


def kernel(**inputs):
    out, _ = run(inputs, trace=False)
    return out
